# revision 3
# baseline (speedup 1.0000x reference)
"""Longformer sliding-window attention on 8 trn2 NeuronCores.

B=2, H=12, L=4096, D=64, one-sided window w=256 (full window 513).
Shard: 24 (b,h) pairs -> 3 heads per core.

Per-core algorithm (per head, 32 key-blocks of 128 keys):
  S^T: per key block, scores (128 keys x <=640 queries) via bf16 matmuls
       (lhsT = K^T block, rhs = Q^T span) into a [128, 640] PSUM tile
       (2 banks, triple-buffered), split at the PSUM bank boundary.
  P^T = exp(S/8) -> SBUF bf16. Most blocks run on ScalarE (activation);
       a spread subset runs as a one-op Schraudolph bit-trick on VectorE
       (fused scale+bias with f32->int16 output convert; the int16 bits
       ARE the bf16 probabilities) to balance the two engines. Band-edge
       triangles masked multiplicatively on VectorE (one strided op per
       block covering both 128x128 corners).
  PV:  matmul with lhsT = [V64 | ones64] per key block: out rows 0:64 =
       unnormalized ctx^T, rows 64:128 = softmax denominator replicas,
       accumulated into [128, 512] PSUM tiles (2 banks, double-buffered).
  evac: DVE copy rows 0:65 PSUM -> SBUF outbuf; outputs (ctx^T + denom
       row) ship eagerly per tile-pair. Softmax division happens on host.

All inputs bf16 (host-cast): Q^T/K^T packed on the same 64 partitions
(one tensor, DMA'd in quarters so the first blocks start early), V
interleaved [V|ones] per block on host. QK runs two blocks ahead of its
consumers to keep ACT/DVE fed; PE p-state warmup matmuls run during the
first input DMA. Non-final-head output DMAs issue from GpSimd (SWDGE)
so they cannot block input DMAs on the SP queue; the final head issues
per-tile from SP (idle at the tail, lower latency than SWDGE).
"""

import sys

sys.path.insert(0, "/opt/trn_rl_repo")

import numpy as np
import ml_dtypes

B, H, L, D = 2, 12, 4096, 64
W = 256            # one-sided window
NCORES = 8
HPC = (B * H) // NCORES   # heads per core = 3
BLK = 128                 # key block (partition dim)
NB = L // BLK             # 32 key blocks per head
SPAN = 2 * W + BLK        # 640 query columns per key block
CTXW = 512                # ctx psum tile width (1 bank)
NT = L // CTXW            # 8 ctx tiles per head

_CACHE = {}
REPEAT = 1           # duplicate compute body for wall-clock timing

# Key blocks whose exp runs as a one-op Schraudolph bit-trick on VectorE
# instead of ScalarE, rebalancing the two engines (ACT is the bottleneck):
# pt_bits_int16 = rint(score * 128*log2(e)/8 + 128*(127 - corr)), viewed
# as bf16. ~2-4% relative error on those probabilities; softmax-normalized
# output error stays ~1e-2 (tolerance 2e-2). Full-span middle blocks only.
DVE_EXP_BLOCKS = frozenset((3, 7, 11, 15, 19, 23, 27))
SCHRAUDOLPH_C1 = 128.0 / np.log(2.0) / 8.0
SCHRAUDOLPH_C2 = 128.0 * (127.0 - 0.0579)

QK_AHEAD = 2         # blocks of QK lookahead (st pool depth - 1)


def _geometry():
    geo = []
    for kb in range(NB):
        K0 = BLK * kb
        qbase = K0 - W
        qlo = max(0, qbase)
        qhi = min(L, K0 + BLK + W)
        geo.append((K0, qbase, qlo, qhi))
    contrib = {t: [] for t in range(NT)}
    for kb, (K0, qbase, qlo, qhi) in enumerate(geo):
        for t in range(qlo // CTXW, (qhi - 1) // CTXW + 1):
            contrib[t].append(kb)
    last_kb = {t: kbs[-1] for t, kbs in contrib.items()}
    return geo, last_kb


def _build_program():
    import concourse.bacc as bacc
    import concourse.bass as bass
    import concourse.mybir as mybir
    import concourse.tile as tile

    f32 = mybir.dt.float32
    bf16 = mybir.dt.bfloat16

    nc = bacc.Bacc("TRN2", target_bir_lowering=False, debug=False)

    qk_d = nc.dram_tensor("qk", [HPC, D, 2 * L], bf16, kind="ExternalInput").ap()
    v_d = nc.dram_tensor("v", [HPC, BLK, NB * 2 * D], bf16, kind="ExternalInput").ap()
    em_d = nc.dram_tensor("masks", [BLK, 2, BLK], bf16, kind="ExternalInput").ap()
    out_d = nc.dram_tensor("out", [HPC, D + 1, L], f32, kind="ExternalOutput").ap()

    geo, last_kb = _geometry()

    with tile.TileContext(nc) as tc:
        with (
            tc.tile_pool(name="const", bufs=1) as constp,
            tc.tile_pool(name="qk", bufs=HPC + 1) as qkp,
            tc.tile_pool(name="v2", bufs=HPC + 1) as v2p,
            tc.tile_pool(name="outb", bufs=HPC) as outp,
            tc.tile_pool(name="pt", bufs=5) as ptp,
            tc.tile_pool(name="st", bufs=QK_AHEAD + 1, space="PSUM") as stp,
            tc.tile_pool(name="ctx", bufs=2, space="PSUM") as ctxp,
        ):
            em = constp.tile([BLK, 2, BLK], bf16)
            # zeroed scratch for PE p-state warmup matmuls
            ws = constp.tile([D, 512], bf16)
            nc.gpsimd.memset(ws, 0.0)

            first = True
            heads = [hh for _ in range(REPEAT) for hh in range(HPC)]
            for hi, h in enumerate(heads):
                last_head = hi == len(heads) - 1
                # packed [Q^T | K^T] on the same 64 partitions; DMA split in
                # quarters (interleaved with V halves) so the first blocks'
                # inputs land as early as possible
                qk = qkp.tile([D, 2, L], bf16, name="qk_t", tag="qk_t")
                qk_src = qk_d[h].rearrange("p (s l) -> p s l", s=2)
                # per-block [V64 | ones64] PV weights, interleaved on host
                v2 = v2p.tile([BLK, NB, 2 * D], bf16, name="v2_t", tag="v2_t")
                v_src = v_d[h].rearrange("p (nb d) -> p nb d", nb=NB)

                Q = L // 4
                nc.sync.dma_start(out=qk[:, :, 0:Q], in_=qk_src[:, :, 0:Q])
                if first:
                    nc.sync.dma_start(out=em, in_=em_d)
                    first = False
                nc.sync.dma_start(out=v2[:, 0 : NB // 2], in_=v_src[:, 0 : NB // 2])
                nc.sync.dma_start(out=qk[:, :, Q : 2 * Q], in_=qk_src[:, :, Q : 2 * Q])
                nc.sync.dma_start(
                    out=qk[:, :, 2 * Q : 3 * Q], in_=qk_src[:, :, 2 * Q : 3 * Q]
                )
                nc.sync.dma_start(out=v2[:, NB // 2 :], in_=v_src[:, NB // 2 :])
                nc.sync.dma_start(out=qk[:, :, 3 * Q :], in_=qk_src[:, :, 3 * Q :])
                qt = qk[:, 0]
                kt = qk[:, 1]

                outbuf = outp.tile([D + 1, L], f32, name="outb_t", tag="outb_t")

                ctx_tiles = {}
                ctx_started = set()
                sts = {}

                def emit_qk(kb):
                    st = stp.tile([BLK, SPAN], f32, name="st_t", tag="st_t")
                    if hi == 0 and kb == 0:
                        # ramp the PE p-state while the first input DMA is in
                        # flight; results land in a bank the real QK clears
                        # with start=True
                        for _ in range(7):
                            nc.tensor.matmul(
                                st[:, 0:512], ws[:, 0:BLK], ws, start=True, stop=True
                            )
                    K0, qbase, qlo, qhi = geo[kb]
                    a, b = qlo - qbase, qhi - qbase
                    lhsT = kt[:, K0 : K0 + BLK]
                    if a < 512:
                        m = min(512, b)
                        nc.tensor.matmul(
                            st[:, a:m],
                            lhsT,
                            qt[:, qbase + a : qbase + m],
                            start=True,
                            stop=True,
                        )
                    if b > 512:
                        nc.tensor.matmul(
                            st[:, 512:b],
                            lhsT,
                            qt[:, qbase + 512 : qbase + b],
                            start=True,
                            stop=True,
                        )
                    sts[kb] = (st, a, b)

                def emit_evac(t):
                    ct = ctx_tiles.pop(t)
                    nc.vector.tensor_copy(
                        outbuf[:, CTXW * t : CTXW * (t + 1)], ct[0 : D + 1, :]
                    )
                    # output DMAs issue from GpSimd (SWDGE) so they never
                    # block later heads' input DMAs on SP. The final head
                    # uses SP per-tile instead: SP is idle at the tail and
                    # HWDGE avoids the ~1us Q7 descriptor-gen on the
                    # exposed chain.
                    if last_head:
                        c0, c1 = CTXW * t, CTXW * (t + 1)
                        nc.sync.dma_start(out=out_d[h][:, c0:c1], in_=outbuf[:, c0:c1])
                    elif t % 2 == 1:
                        c0, c1 = CTXW * (t - 1), CTXW * (t + 1)
                        nc.gpsimd.dma_start(
                            out=out_d[h][:, c0:c1], in_=outbuf[:, c0:c1]
                        )

                for kb in range(QK_AHEAD):
                    emit_qk(kb)
                pending_evac = []
                scale = float(1.0 / np.sqrt(D))
                Exp = mybir.ActivationFunctionType.Exp
                for kb in range(NB):
                    if kb + QK_AHEAD < NB:
                        emit_qk(kb + QK_AHEAD)  # QK ahead: PE priority bias
                    st, a, b = sts.pop(kb)
                    K0, qbase, qlo, qhi = geo[kb]
                    pt = ptp.tile([BLK, SPAN], bf16, name="pt_t", tag="pt_t")

                    # exp(S/8)
                    if kb in DVE_EXP_BLOCKS:
                        # Schraudolph exp on DVE: fused scale+bias with
                        # f32->int16 output convert; int16 bits ARE the bf16
                        # probabilities
                        nc.vector.tensor_scalar(
                            pt.bitcast(mybir.dt.int16),
                            st,
                            float(SCHRAUDOLPH_C1),
                            float(SCHRAUDOLPH_C2),
                            mybir.AluOpType.mult,
                            mybir.AluOpType.add,
                        )
                    else:
                        nc.scalar.activation(pt[:, a:b], st[:, a:b], Exp, scale=scale)

                    # triangle masks on the band edges (multiplicative);
                    # middle blocks hit both corners in one strided op,
                    # alternating DVE / GpSimd to spread the load
                    if a == 0 and b == SPAN:
                        pte = bass.AP(
                            tensor=pt.tensor,
                            offset=pt.offset,
                            ap=[pt.ap[0], [4 * BLK, 2], [1, BLK]],
                        )
                        nc.vector.tensor_mul(pte, pte, em)
                    elif a == 0:
                        nc.vector.tensor_mul(pt[:, 0:BLK], pt[:, 0:BLK], em[:, 0, :])
                    elif b == SPAN:
                        nc.vector.tensor_mul(
                            pt[:, 512:SPAN], pt[:, 512:SPAN], em[:, 1, :]
                        )

                    # evacs deferred one block: they wait on PV, and emitting
                    # them here keeps them from head-of-line-blocking this
                    # block's mask in DVE's in-order queue
                    for t in pending_evac:
                        emit_evac(t)
                    pending_evac = []

                    # PV accumulation into ctx tiles
                    lhsT = v2[:, kb, :]
                    for t in range(qlo // CTXW, (qhi - 1) // CTXW + 1):
                        ta = max(qlo, CTXW * t)
                        tb = min(qhi, CTXW * (t + 1))
                        if t not in ctx_tiles:
                            ctx_tiles[t] = ctxp.tile(
                                [BLK, CTXW], f32, name="ctx_t", tag="ctx_t"
                            )
                        first_mm = t not in ctx_started
                        ctx_started.add(t)
                        nc.tensor.matmul(
                            ctx_tiles[t][:, ta - CTXW * t : tb - CTXW * t],
                            lhsT,
                            pt[:, ta - qbase : tb - qbase],
                            start=first_mm,
                            stop=(kb == last_kb[t]),
                        )

                    for t in list(ctx_tiles):
                        if last_kb[t] == kb:
                            pending_evac.append(t)

                # flush remaining evacs at head end
                for t in pending_evac:
                    emit_evac(t)
                pending_evac = []

    nc.compile()
    return nc


def _get_nc():
    if "nc" not in _CACHE:
        _CACHE["nc"] = _build_program()
    return _CACHE["nc"]


def _host_prep(q, k, v):
    bf = ml_dtypes.bfloat16
    qf = np.asarray(q, dtype=np.float32).reshape(B * H, L, D)
    kf = np.asarray(k, dtype=np.float32).reshape(B * H, L, D)
    # packed [Q^T | K^T] on the same 64 partitions: (BH, 64, 2*L)
    qk = np.empty((B * H, D, 2 * L), dtype=bf)
    qk[:, :, 0:L] = qf.transpose(0, 2, 1).astype(bf)
    qk[:, :, L : 2 * L] = kf.transpose(0, 2, 1).astype(bf)
    # PV weights per key block: [V64 | ones64], (BH, BLK, NB*2D) with
    # [r, kb, d] = v[kb*BLK + r, d] and [r, kb, D:] = 1
    vf = np.ones((B * H, BLK, NB, 2 * D), dtype=bf)
    vf[:, :, :, 0:D] = (
        np.asarray(v, dtype=np.float32)
        .reshape(B * H, NB, BLK, D)
        .transpose(0, 2, 1, 3)
        .astype(bf)
    )
    vf = vf.reshape(B * H, BLK, NB * 2 * D)

    i = np.arange(BLK)
    em = np.zeros((BLK, 2, BLK), dtype=bf)
    em[:, 0, :] = (i[None, :] >= i[:, None]).astype(bf)  # left: col>=row
    em[:, 1, :] = (i[None, :] <= i[:, None]).astype(bf)  # right: col<=row

    in_maps = []
    for c in range(NCORES):
        sl = slice(c * HPC, (c + 1) * HPC)
        in_maps.append(
            {
                "qk": np.ascontiguousarray(qk[sl]),
                "v": np.ascontiguousarray(vf[sl]),
                "masks": em,
            }
        )
    return in_maps


def kernel(q, k, v, padding_mask):
    from concourse.bass_utils import run_bass_kernel_spmd

    pm = np.asarray(padding_mask)
    assert pm.all(), "kernel specialized for all-ones padding mask"

    nc = _get_nc()
    in_maps = _host_prep(q, k, v)
    try:
        res = run_bass_kernel_spmd(nc, in_maps, core_ids=list(range(NCORES)))
    except Exception:
        # transient NRT_EXEC_UNIT_UNRECOVERABLE has been observed once on
        # this axon setup; the identical program+inputs passed on retry
        res = run_bass_kernel_spmd(nc, in_maps, core_ids=list(range(NCORES)))
    both = np.concatenate(
        [res.results[c]["out"] for c in range(NCORES)], axis=0
    )  # (24, 65, 4096): rows 0:64 unnormalized ctx^T, row 64 denominator
    full = both[:, 0:D, :] / both[:, D : D + 1, :]  # softmax norm on host
    out = full.transpose(0, 2, 1).reshape(B, H, L, D)
    return np.ascontiguousarray(out.astype(np.float32))


# revision 4
# speedup vs baseline: 1.0046x; 1.0046x over previous
"""Longformer sliding-window attention on 8 trn2 NeuronCores.

B=2, H=12, L=4096, D=64, one-sided window w=256 (full window 513).
Shard: 24 (b,h) pairs -> 3 heads per core.

Per-core algorithm (per head, 32 key-blocks of 128 keys):
  S^T: per key block, scores (128 keys x <=640 queries) via bf16 matmuls
       (lhsT = K^T block, rhs = Q^T span) into a [128, 640] PSUM tile
       (2 banks, triple-buffered), split at the PSUM bank boundary.
  P^T = exp(S/8) -> SBUF bf16. Most blocks run on ScalarE (activation);
       a spread subset runs as a one-op Schraudolph bit-trick on VectorE
       (fused scale+bias with f32->int16 output convert; the int16 bits
       ARE the bf16 probabilities) to balance the two engines. Band-edge
       triangles masked multiplicatively on VectorE (one strided op per
       block covering both 128x128 corners).
  PV:  matmul with lhsT = [V64 | ones64] per key block: out rows 0:64 =
       unnormalized ctx^T, rows 64:128 = softmax denominator replicas,
       accumulated into [128, 512] PSUM tiles (2 banks, double-buffered).
  evac: DVE copy rows 0:65 PSUM -> SBUF outbuf; outputs (ctx^T + denom
       row) ship eagerly per tile-pair. Softmax division happens on host.

All inputs bf16 (host-cast): Q^T/K^T packed on the same 64 partitions
(one tensor, DMA'd in quarters so the first blocks start early), V
interleaved [V|ones] per block on host. QK runs two blocks ahead of its
consumers to keep ACT/DVE fed; PE p-state warmup matmuls run during the
first input DMA. Non-final-head output DMAs issue from GpSimd (SWDGE)
so they cannot block input DMAs on the SP queue; the final head issues
per-tile from SP (idle at the tail, lower latency than SWDGE).
"""

import sys

sys.path.insert(0, "/opt/trn_rl_repo")

import numpy as np
import ml_dtypes

B, H, L, D = 2, 12, 4096, 64
W = 256            # one-sided window
NCORES = 8
HPC = (B * H) // NCORES   # heads per core = 3
BLK = 128                 # key block (partition dim)
NB = L // BLK             # 32 key blocks per head
SPAN = 2 * W + BLK        # 640 query columns per key block
CTXW = 512                # ctx psum tile width (1 bank)
NT = L // CTXW            # 8 ctx tiles per head

_CACHE = {}
REPEAT = 1           # duplicate compute body for wall-clock timing

# Key blocks whose exp runs as a one-op Schraudolph bit-trick on VectorE
# instead of ScalarE, rebalancing the two engines (ACT is the bottleneck):
# pt_bits_int16 = rint(score * 128*log2(e)/8 + 128*(127 - corr)), viewed
# as bf16. ~2-4% relative error on those probabilities; softmax-normalized
# output error stays ~1e-2 (tolerance 2e-2). Full-span middle blocks only.
DVE_EXP_BLOCKS = frozenset((3, 7, 11, 15, 19, 23, 27))
SCHRAUDOLPH_C1 = 128.0 / np.log(2.0) / 8.0
SCHRAUDOLPH_C2 = 128.0 * (127.0 - 0.0579)

QK_AHEAD = 2         # blocks of QK lookahead (st pool depth - 1)


def _geometry():
    geo = []
    for kb in range(NB):
        K0 = BLK * kb
        qbase = K0 - W
        qlo = max(0, qbase)
        qhi = min(L, K0 + BLK + W)
        geo.append((K0, qbase, qlo, qhi))
    contrib = {t: [] for t in range(NT)}
    for kb, (K0, qbase, qlo, qhi) in enumerate(geo):
        for t in range(qlo // CTXW, (qhi - 1) // CTXW + 1):
            contrib[t].append(kb)
    last_kb = {t: kbs[-1] for t, kbs in contrib.items()}
    return geo, last_kb


def _build_program():
    import concourse.bacc as bacc
    import concourse.bass as bass
    import concourse.mybir as mybir
    import concourse.tile as tile

    f32 = mybir.dt.float32
    bf16 = mybir.dt.bfloat16

    nc = bacc.Bacc("TRN2", target_bir_lowering=False, debug=False)

    qk_d = nc.dram_tensor("qk", [HPC, D, 2 * L], bf16, kind="ExternalInput").ap()
    v_d = nc.dram_tensor("v", [HPC, BLK, NB * 2 * D], bf16, kind="ExternalInput").ap()
    em_d = nc.dram_tensor("masks", [BLK, 2, BLK], bf16, kind="ExternalInput").ap()
    out_d = nc.dram_tensor("out", [HPC, D + 1, L], f32, kind="ExternalOutput").ap()

    geo, last_kb = _geometry()

    with tile.TileContext(nc) as tc:
        with (
            tc.tile_pool(name="const", bufs=1) as constp,
            tc.tile_pool(name="qk", bufs=HPC + 1) as qkp,
            tc.tile_pool(name="v2", bufs=HPC + 1) as v2p,
            tc.tile_pool(name="outb", bufs=HPC) as outp,
            tc.tile_pool(name="pt", bufs=5) as ptp,
            tc.tile_pool(name="st", bufs=QK_AHEAD + 1, space="PSUM") as stp,
            tc.tile_pool(name="ctx", bufs=2, space="PSUM") as ctxp,
        ):
            em = constp.tile([BLK, 2, BLK], bf16)
            # zeroed scratch for PE p-state warmup matmuls
            ws = constp.tile([D, 512], bf16)
            nc.gpsimd.memset(ws, 0.0)

            first = True
            heads = [hh for _ in range(REPEAT) for hh in range(HPC)]
            for hi, h in enumerate(heads):
                last_head = hi == len(heads) - 1
                # packed [Q^T | K^T] on the same 64 partitions; DMA split in
                # quarters (interleaved with V halves) so the first blocks'
                # inputs land as early as possible
                qk = qkp.tile([D, 2, L], bf16, name="qk_t", tag="qk_t")
                qk_src = qk_d[h].rearrange("p (s l) -> p s l", s=2)
                # per-block [V64 | ones64] PV weights, interleaved on host
                v2 = v2p.tile([BLK, NB, 2 * D], bf16, name="v2_t", tag="v2_t")
                v_src = v_d[h].rearrange("p (nb d) -> p nb d", nb=NB)

                Q = L // 4
                nc.sync.dma_start(out=qk[:, :, 0:Q], in_=qk_src[:, :, 0:Q])
                if first:
                    nc.sync.dma_start(out=em, in_=em_d)
                    first = False
                nc.sync.dma_start(out=v2[:, 0 : NB // 2], in_=v_src[:, 0 : NB // 2])
                nc.sync.dma_start(out=qk[:, :, Q : 2 * Q], in_=qk_src[:, :, Q : 2 * Q])
                nc.sync.dma_start(
                    out=qk[:, :, 2 * Q : 3 * Q], in_=qk_src[:, :, 2 * Q : 3 * Q]
                )
                nc.sync.dma_start(out=v2[:, NB // 2 :], in_=v_src[:, NB // 2 :])
                nc.sync.dma_start(out=qk[:, :, 3 * Q :], in_=qk_src[:, :, 3 * Q :])
                qt = qk[:, 0]
                kt = qk[:, 1]

                outbuf = outp.tile([D + 1, L], f32, name="outb_t", tag="outb_t")

                ctx_tiles = {}
                ctx_started = set()
                sts = {}

                def emit_qk(kb):
                    st = stp.tile([BLK, SPAN], f32, name="st_t", tag="st_t")
                    if hi == 0 and kb == 0:
                        # ramp the PE p-state while the first input DMA is in
                        # flight; results land in a bank the real QK clears
                        # with start=True
                        for _ in range(7):
                            nc.tensor.matmul(
                                st[:, 0:512], ws[:, 0:BLK], ws, start=True, stop=True
                            )
                    K0, qbase, qlo, qhi = geo[kb]
                    a, b = qlo - qbase, qhi - qbase
                    lhsT = kt[:, K0 : K0 + BLK]
                    if a < 512:
                        m = min(512, b)
                        nc.tensor.matmul(
                            st[:, a:m],
                            lhsT,
                            qt[:, qbase + a : qbase + m],
                            start=True,
                            stop=True,
                        )
                    if b > 512:
                        nc.tensor.matmul(
                            st[:, 512:b],
                            lhsT,
                            qt[:, qbase + 512 : qbase + b],
                            start=True,
                            stop=True,
                        )
                    sts[kb] = (st, a, b)

                def emit_evac(t):
                    ct = ctx_tiles.pop(t)
                    nc.vector.tensor_copy(
                        outbuf[:, CTXW * t : CTXW * (t + 1)], ct[0 : D + 1, :]
                    )
                    # output DMAs issue from GpSimd (SWDGE) so they never
                    # block later heads' input DMAs on SP. The final head
                    # uses SP per-tile instead: SP is idle at the tail and
                    # HWDGE avoids the ~1us Q7 descriptor-gen on the
                    # exposed chain.
                    if last_head:
                        c0, c1 = CTXW * t, CTXW * (t + 1)
                        nc.sync.dma_start(out=out_d[h][:, c0:c1], in_=outbuf[:, c0:c1])
                    elif t % 2 == 1:
                        c0, c1 = CTXW * (t - 1), CTXW * (t + 1)
                        nc.gpsimd.dma_start(
                            out=out_d[h][:, c0:c1], in_=outbuf[:, c0:c1]
                        )

                for kb in range(QK_AHEAD):
                    emit_qk(kb)
                pending_evac = []
                scale = float(1.0 / np.sqrt(D))
                Exp = mybir.ActivationFunctionType.Exp
                for kb in range(NB):
                    if kb + QK_AHEAD < NB:
                        emit_qk(kb + QK_AHEAD)  # QK ahead: PE priority bias
                    st, a, b = sts.pop(kb)
                    K0, qbase, qlo, qhi = geo[kb]
                    pt = ptp.tile([BLK, SPAN], bf16, name="pt_t", tag="pt_t")

                    # exp(S/8)
                    if (kb - hi) % 32 in DVE_EXP_BLOCKS and 2 <= kb <= 29:
                        # Schraudolph exp on DVE: fused scale+bias with
                        # f32->int16 output convert; int16 bits ARE the bf16
                        # probabilities
                        nc.vector.tensor_scalar(
                            pt.bitcast(mybir.dt.int16),
                            st,
                            float(SCHRAUDOLPH_C1),
                            float(SCHRAUDOLPH_C2),
                            mybir.AluOpType.mult,
                            mybir.AluOpType.add,
                        )
                    else:
                        nc.scalar.activation(pt[:, a:b], st[:, a:b], Exp, scale=scale)

                    # triangle masks on the band edges (multiplicative);
                    # middle blocks hit both corners in one strided op,
                    # alternating DVE / GpSimd to spread the load
                    if a == 0 and b == SPAN:
                        pte = bass.AP(
                            tensor=pt.tensor,
                            offset=pt.offset,
                            ap=[pt.ap[0], [4 * BLK, 2], [1, BLK]],
                        )
                        nc.vector.tensor_mul(pte, pte, em)
                    elif a == 0:
                        nc.vector.tensor_mul(pt[:, 0:BLK], pt[:, 0:BLK], em[:, 0, :])
                    elif b == SPAN:
                        nc.vector.tensor_mul(
                            pt[:, 512:SPAN], pt[:, 512:SPAN], em[:, 1, :]
                        )

                    # evacs deferred one block: they wait on PV, and emitting
                    # them here keeps them from head-of-line-blocking this
                    # block's mask in DVE's in-order queue
                    for t in pending_evac:
                        emit_evac(t)
                    pending_evac = []

                    # PV accumulation into ctx tiles
                    lhsT = v2[:, kb, :]
                    for t in range(qlo // CTXW, (qhi - 1) // CTXW + 1):
                        ta = max(qlo, CTXW * t)
                        tb = min(qhi, CTXW * (t + 1))
                        if t not in ctx_tiles:
                            ctx_tiles[t] = ctxp.tile(
                                [BLK, CTXW], f32, name="ctx_t", tag="ctx_t"
                            )
                        first_mm = t not in ctx_started
                        ctx_started.add(t)
                        nc.tensor.matmul(
                            ctx_tiles[t][:, ta - CTXW * t : tb - CTXW * t],
                            lhsT,
                            pt[:, ta - qbase : tb - qbase],
                            start=first_mm,
                            stop=(kb == last_kb[t]),
                        )

                    for t in list(ctx_tiles):
                        if last_kb[t] == kb:
                            pending_evac.append(t)

                # flush remaining evacs at head end
                for t in pending_evac:
                    emit_evac(t)
                pending_evac = []

    nc.compile()
    return nc


def _get_nc():
    if "nc" not in _CACHE:
        _CACHE["nc"] = _build_program()
    return _CACHE["nc"]


def _host_prep(q, k, v):
    bf = ml_dtypes.bfloat16
    qf = np.asarray(q, dtype=np.float32).reshape(B * H, L, D)
    kf = np.asarray(k, dtype=np.float32).reshape(B * H, L, D)
    # packed [Q^T | K^T] on the same 64 partitions: (BH, 64, 2*L)
    qk = np.empty((B * H, D, 2 * L), dtype=bf)
    qk[:, :, 0:L] = qf.transpose(0, 2, 1).astype(bf)
    qk[:, :, L : 2 * L] = kf.transpose(0, 2, 1).astype(bf)
    # PV weights per key block: [V64 | ones64], (BH, BLK, NB*2D) with
    # [r, kb, d] = v[kb*BLK + r, d] and [r, kb, D:] = 1
    vf = np.ones((B * H, BLK, NB, 2 * D), dtype=bf)
    vf[:, :, :, 0:D] = (
        np.asarray(v, dtype=np.float32)
        .reshape(B * H, NB, BLK, D)
        .transpose(0, 2, 1, 3)
        .astype(bf)
    )
    vf = vf.reshape(B * H, BLK, NB * 2 * D)

    i = np.arange(BLK)
    em = np.zeros((BLK, 2, BLK), dtype=bf)
    em[:, 0, :] = (i[None, :] >= i[:, None]).astype(bf)  # left: col>=row
    em[:, 1, :] = (i[None, :] <= i[:, None]).astype(bf)  # right: col<=row

    in_maps = []
    for c in range(NCORES):
        sl = slice(c * HPC, (c + 1) * HPC)
        in_maps.append(
            {
                "qk": np.ascontiguousarray(qk[sl]),
                "v": np.ascontiguousarray(vf[sl]),
                "masks": em,
            }
        )
    return in_maps


def kernel(q, k, v, padding_mask):
    from concourse.bass_utils import run_bass_kernel_spmd

    pm = np.asarray(padding_mask)
    assert pm.all(), "kernel specialized for all-ones padding mask"

    nc = _get_nc()
    in_maps = _host_prep(q, k, v)
    try:
        res = run_bass_kernel_spmd(nc, in_maps, core_ids=list(range(NCORES)))
    except Exception:
        # transient NRT_EXEC_UNIT_UNRECOVERABLE has been observed once on
        # this axon setup; the identical program+inputs passed on retry
        res = run_bass_kernel_spmd(nc, in_maps, core_ids=list(range(NCORES)))
    both = np.concatenate(
        [res.results[c]["out"] for c in range(NCORES)], axis=0
    )  # (24, 65, 4096): rows 0:64 unnormalized ctx^T, row 64 denominator
    full = both[:, 0:D, :] / both[:, D : D + 1, :]  # softmax norm on host
    out = full.transpose(0, 2, 1).reshape(B, H, L, D)
    return np.ascontiguousarray(out.astype(np.float32))


# revision 5
# speedup vs baseline: 1.0139x; 1.0092x over previous
"""Longformer sliding-window attention on 8 trn2 NeuronCores.

B=2, H=12, L=4096, D=64, one-sided window w=256 (full window 513).
Shard: 24 (b,h) pairs -> 3 heads per core.

Per-core algorithm (per head, 32 key-blocks of 128 keys):
  S^T: per key block, scores (128 keys x <=640 queries) via bf16 matmuls
       (lhsT = K^T block, rhs = Q^T span) into a [128, 640] PSUM tile
       (2 banks, triple-buffered), split at the PSUM bank boundary.
  P^T = exp(S/8) -> SBUF bf16. Most blocks run on ScalarE (activation);
       a spread subset runs as a one-op Schraudolph bit-trick on VectorE
       (fused scale+bias with f32->int16 output convert; the int16 bits
       ARE the bf16 probabilities) to balance the two engines. Band-edge
       triangles masked multiplicatively on VectorE (one strided op per
       block covering both 128x128 corners).
  PV:  matmul with lhsT = [V64 | ones64] per key block: out rows 0:64 =
       unnormalized ctx^T, rows 64:128 = softmax denominator replicas,
       accumulated into [128, 512] PSUM tiles (2 banks, double-buffered).
  evac: DVE copy rows 0:65 PSUM -> SBUF outbuf; outputs (ctx^T + denom
       row) ship eagerly per tile-pair. Softmax division happens on host.

All inputs bf16 (host-cast): Q^T/K^T packed on the same 64 partitions
(one tensor, DMA'd in quarters so the first blocks start early), V
interleaved [V|ones] per block on host. QK runs two blocks ahead of its
consumers to keep ACT/DVE fed; PE p-state warmup matmuls run during the
first input DMA. Non-final-head output DMAs issue from GpSimd (SWDGE)
so they cannot block input DMAs on the SP queue; the final head issues
per-tile from SP (idle at the tail, lower latency than SWDGE).
"""

import sys

sys.path.insert(0, "/opt/trn_rl_repo")

import numpy as np
import ml_dtypes

B, H, L, D = 2, 12, 4096, 64
W = 256            # one-sided window
NCORES = 8
HPC = (B * H) // NCORES   # heads per core = 3
BLK = 128                 # key block (partition dim)
NB = L // BLK             # 32 key blocks per head
SPAN = 2 * W + BLK        # 640 query columns per key block
CTXW = 512                # ctx psum tile width (1 bank)
NT = L // CTXW            # 8 ctx tiles per head

_CACHE = {}
REPEAT = 1           # duplicate compute body for wall-clock timing

# Key blocks whose exp runs as a one-op Schraudolph bit-trick on VectorE
# instead of ScalarE, rebalancing the two engines (ACT is the bottleneck):
# pt_bits_int16 = rint(score * 128*log2(e)/8 + 128*(127 - corr)), viewed
# as bf16. ~2-4% relative error on those probabilities; softmax-normalized
# output error stays ~1e-2 (tolerance 2e-2). Full-span middle blocks only.
DVE_EXP_BLOCKS = frozenset((3, 7, 11, 15, 19, 23, 27))
SCHRAUDOLPH_C1 = 128.0 / np.log(2.0) / 8.0
SCHRAUDOLPH_C2 = 128.0 * (127.0 - 0.0579)

QK_AHEAD = 2         # blocks of QK lookahead (st pool depth - 1)


def _geometry():
    geo = []
    for kb in range(NB):
        K0 = BLK * kb
        qbase = K0 - W
        qlo = max(0, qbase)
        qhi = min(L, K0 + BLK + W)
        geo.append((K0, qbase, qlo, qhi))
    contrib = {t: [] for t in range(NT)}
    for kb, (K0, qbase, qlo, qhi) in enumerate(geo):
        for t in range(qlo // CTXW, (qhi - 1) // CTXW + 1):
            contrib[t].append(kb)
    last_kb = {t: kbs[-1] for t, kbs in contrib.items()}
    return geo, last_kb


def _build_program():
    import concourse.bacc as bacc
    import concourse.bass as bass
    import concourse.mybir as mybir
    import concourse.tile as tile

    f32 = mybir.dt.float32
    bf16 = mybir.dt.bfloat16

    nc = bacc.Bacc("TRN2", target_bir_lowering=False, debug=False)

    qk_d = nc.dram_tensor("qk", [HPC, D, 2 * L], bf16, kind="ExternalInput").ap()
    v_d = nc.dram_tensor("v", [HPC, BLK, NB * 2 * D], bf16, kind="ExternalInput").ap()
    em_d = nc.dram_tensor("masks", [BLK, 2, BLK], bf16, kind="ExternalInput").ap()
    out_d = nc.dram_tensor("out", [HPC, D + 1, L], f32, kind="ExternalOutput").ap()

    geo, last_kb = _geometry()

    with tile.TileContext(nc) as tc:
        with (
            tc.tile_pool(name="const", bufs=1) as constp,
            tc.tile_pool(name="qk", bufs=HPC + 1) as qkp,
            tc.tile_pool(name="v2", bufs=HPC + 1) as v2p,
            tc.tile_pool(name="outb", bufs=HPC) as outp,
            tc.tile_pool(name="pt", bufs=5) as ptp,
            tc.tile_pool(name="st", bufs=QK_AHEAD + 1, space="PSUM") as stp,
            tc.tile_pool(name="ctx", bufs=2, space="PSUM") as ctxp,
        ):
            em = constp.tile([BLK, 2, BLK], bf16)
            # zeroed scratch for PE p-state warmup matmuls
            ws = constp.tile([D, 512], bf16)
            nc.gpsimd.memset(ws, 0.0)

            first = True
            heads = [hh for _ in range(REPEAT) for hh in range(HPC)]
            for hi, h in enumerate(heads):
                last_head = hi == len(heads) - 1
                # packed [Q^T | K^T] on the same 64 partitions; DMA split in
                # quarters (interleaved with V halves) so the first blocks'
                # inputs land as early as possible
                qk = qkp.tile([D, 2, L], bf16, name="qk_t", tag="qk_t")
                qk_src = qk_d[h].rearrange("p (s l) -> p s l", s=2)
                # per-block [V64 | ones64] PV weights, interleaved on host
                v2 = v2p.tile([BLK, NB, 2 * D], bf16, name="v2_t", tag="v2_t")
                v_src = v_d[h].rearrange("p (nb d) -> p nb d", nb=NB)

                Q = L // 4
                nc.sync.dma_start(out=qk[:, :, 0:Q], in_=qk_src[:, :, 0:Q])
                if first:
                    nc.sync.dma_start(out=em, in_=em_d)
                    first = False
                nc.sync.dma_start(out=v2[:, 0 : NB // 2], in_=v_src[:, 0 : NB // 2])
                nc.sync.dma_start(out=qk[:, :, Q : 2 * Q], in_=qk_src[:, :, Q : 2 * Q])
                nc.sync.dma_start(
                    out=qk[:, :, 2 * Q : 3 * Q], in_=qk_src[:, :, 2 * Q : 3 * Q]
                )
                nc.sync.dma_start(out=v2[:, NB // 2 :], in_=v_src[:, NB // 2 :])
                nc.sync.dma_start(out=qk[:, :, 3 * Q :], in_=qk_src[:, :, 3 * Q :])
                qt = qk[:, 0]
                kt = qk[:, 1]

                outbuf = outp.tile([D + 1, L], f32, name="outb_t", tag="outb_t")

                ctx_tiles = {}
                ctx_started = set()
                sts = {}

                def emit_qk(kb):
                    st = stp.tile([BLK, SPAN], f32, name="st_t", tag="st_t")
                    if hi == 0 and kb == 0:
                        # ramp the PE p-state while the first input DMA is in
                        # flight; results land in a bank the real QK clears
                        # with start=True
                        for _ in range(5):
                            nc.tensor.matmul(
                                st[:, 0:512], ws[:, 0:BLK], ws, start=True, stop=True
                            )
                    K0, qbase, qlo, qhi = geo[kb]
                    a, b = qlo - qbase, qhi - qbase
                    lhsT = kt[:, K0 : K0 + BLK]
                    if a < 512:
                        m = min(512, b)
                        nc.tensor.matmul(
                            st[:, a:m],
                            lhsT,
                            qt[:, qbase + a : qbase + m],
                            start=True,
                            stop=True,
                        )
                    if b > 512:
                        nc.tensor.matmul(
                            st[:, 512:b],
                            lhsT,
                            qt[:, qbase + 512 : qbase + b],
                            start=True,
                            stop=True,
                        )
                    sts[kb] = (st, a, b)

                def emit_evac(t):
                    ct = ctx_tiles.pop(t)
                    nc.vector.tensor_copy(
                        outbuf[:, CTXW * t : CTXW * (t + 1)], ct[0 : D + 1, :]
                    )
                    # output DMAs issue from GpSimd (SWDGE) so they never
                    # block later heads' input DMAs on SP. The final head
                    # uses SP per-tile instead: SP is idle at the tail and
                    # HWDGE avoids the ~1us Q7 descriptor-gen on the
                    # exposed chain.
                    if last_head:
                        c0, c1 = CTXW * t, CTXW * (t + 1)
                        nc.sync.dma_start(out=out_d[h][:, c0:c1], in_=outbuf[:, c0:c1])
                    elif t % 2 == 1:
                        c0, c1 = CTXW * (t - 1), CTXW * (t + 1)
                        nc.gpsimd.dma_start(
                            out=out_d[h][:, c0:c1], in_=outbuf[:, c0:c1]
                        )

                for kb in range(QK_AHEAD):
                    emit_qk(kb)
                pending_evac = []
                scale = float(1.0 / np.sqrt(D))
                Exp = mybir.ActivationFunctionType.Exp
                for kb in range(NB):
                    if kb + QK_AHEAD < NB:
                        emit_qk(kb + QK_AHEAD)  # QK ahead: PE priority bias
                    st, a, b = sts.pop(kb)
                    K0, qbase, qlo, qhi = geo[kb]
                    pt = ptp.tile([BLK, SPAN], bf16, name="pt_t", tag="pt_t")

                    # exp(S/8)
                    if (kb - hi) % 32 in DVE_EXP_BLOCKS and 2 <= kb <= 29:
                        # Schraudolph exp on DVE: fused scale+bias with
                        # f32->int16 output convert; int16 bits ARE the bf16
                        # probabilities
                        nc.vector.tensor_scalar(
                            pt.bitcast(mybir.dt.int16),
                            st,
                            float(SCHRAUDOLPH_C1),
                            float(SCHRAUDOLPH_C2),
                            mybir.AluOpType.mult,
                            mybir.AluOpType.add,
                        )
                    else:
                        nc.scalar.activation(pt[:, a:b], st[:, a:b], Exp, scale=scale)

                    # triangle masks on the band edges (multiplicative);
                    # middle blocks hit both corners in one strided op,
                    # alternating DVE / GpSimd to spread the load
                    if a == 0 and b == SPAN:
                        pte = bass.AP(
                            tensor=pt.tensor,
                            offset=pt.offset,
                            ap=[pt.ap[0], [4 * BLK, 2], [1, BLK]],
                        )
                        nc.vector.tensor_mul(pte, pte, em)
                    elif a == 0:
                        nc.vector.tensor_mul(pt[:, 0:BLK], pt[:, 0:BLK], em[:, 0, :])
                    elif b == SPAN:
                        nc.vector.tensor_mul(
                            pt[:, 512:SPAN], pt[:, 512:SPAN], em[:, 1, :]
                        )

                    # evacs deferred one block: they wait on PV, and emitting
                    # them here keeps them from head-of-line-blocking this
                    # block's mask in DVE's in-order queue
                    for t in pending_evac:
                        emit_evac(t)
                    pending_evac = []

                    # PV accumulation into ctx tiles
                    lhsT = v2[:, kb, :]
                    for t in range(qlo // CTXW, (qhi - 1) // CTXW + 1):
                        ta = max(qlo, CTXW * t)
                        tb = min(qhi, CTXW * (t + 1))
                        if t not in ctx_tiles:
                            ctx_tiles[t] = ctxp.tile(
                                [BLK, CTXW], f32, name="ctx_t", tag="ctx_t"
                            )
                        first_mm = t not in ctx_started
                        ctx_started.add(t)
                        nc.tensor.matmul(
                            ctx_tiles[t][:, ta - CTXW * t : tb - CTXW * t],
                            lhsT,
                            pt[:, ta - qbase : tb - qbase],
                            start=first_mm,
                            stop=(kb == last_kb[t]),
                        )

                    for t in list(ctx_tiles):
                        if last_kb[t] == kb:
                            pending_evac.append(t)

                # flush remaining evacs at head end
                for t in pending_evac:
                    emit_evac(t)
                pending_evac = []

    nc.compile()
    return nc


def _get_nc():
    if "nc" not in _CACHE:
        _CACHE["nc"] = _build_program()
    return _CACHE["nc"]


def _host_prep(q, k, v):
    bf = ml_dtypes.bfloat16
    qf = np.asarray(q, dtype=np.float32).reshape(B * H, L, D)
    kf = np.asarray(k, dtype=np.float32).reshape(B * H, L, D)
    # packed [Q^T | K^T] on the same 64 partitions: (BH, 64, 2*L)
    qk = np.empty((B * H, D, 2 * L), dtype=bf)
    qk[:, :, 0:L] = qf.transpose(0, 2, 1).astype(bf)
    qk[:, :, L : 2 * L] = kf.transpose(0, 2, 1).astype(bf)
    # PV weights per key block: [V64 | ones64], (BH, BLK, NB*2D) with
    # [r, kb, d] = v[kb*BLK + r, d] and [r, kb, D:] = 1
    vf = np.ones((B * H, BLK, NB, 2 * D), dtype=bf)
    vf[:, :, :, 0:D] = (
        np.asarray(v, dtype=np.float32)
        .reshape(B * H, NB, BLK, D)
        .transpose(0, 2, 1, 3)
        .astype(bf)
    )
    vf = vf.reshape(B * H, BLK, NB * 2 * D)

    i = np.arange(BLK)
    em = np.zeros((BLK, 2, BLK), dtype=bf)
    em[:, 0, :] = (i[None, :] >= i[:, None]).astype(bf)  # left: col>=row
    em[:, 1, :] = (i[None, :] <= i[:, None]).astype(bf)  # right: col<=row

    in_maps = []
    for c in range(NCORES):
        sl = slice(c * HPC, (c + 1) * HPC)
        in_maps.append(
            {
                "qk": np.ascontiguousarray(qk[sl]),
                "v": np.ascontiguousarray(vf[sl]),
                "masks": em,
            }
        )
    return in_maps


def kernel(q, k, v, padding_mask):
    from concourse.bass_utils import run_bass_kernel_spmd

    pm = np.asarray(padding_mask)
    assert pm.all(), "kernel specialized for all-ones padding mask"

    nc = _get_nc()
    in_maps = _host_prep(q, k, v)
    try:
        res = run_bass_kernel_spmd(nc, in_maps, core_ids=list(range(NCORES)))
    except Exception:
        # transient NRT_EXEC_UNIT_UNRECOVERABLE has been observed once on
        # this axon setup; the identical program+inputs passed on retry
        res = run_bass_kernel_spmd(nc, in_maps, core_ids=list(range(NCORES)))
    both = np.concatenate(
        [res.results[c]["out"] for c in range(NCORES)], axis=0
    )  # (24, 65, 4096): rows 0:64 unnormalized ctx^T, row 64 denominator
    full = both[:, 0:D, :] / both[:, D : D + 1, :]  # softmax norm on host
    out = full.transpose(0, 2, 1).reshape(B, H, L, D)
    return np.ascontiguousarray(out.astype(np.float32))


# revision 6
# speedup vs baseline: 1.1169x; 1.1016x over previous
"""Longformer sliding-window attention on 8 trn2 NeuronCores.

B=2, H=12, L=4096, D=64, one-sided window w=256 (full window 513).
Shard: 24 (b,h) pairs -> 3 heads per core.

Per-core algorithm (per head, 32 key-blocks of 128 keys):
  S^T: per key block, scores (128 keys x <=640 queries) via bf16 matmuls
       (lhsT = K^T block, rhs = Q^T span) into a [128, 640] PSUM tile
       (2 banks, triple-buffered), split at the PSUM bank boundary.
  P^T = exp(S/8) -> SBUF bf16. Most blocks run on ScalarE (activation);
       a spread subset runs as a one-op Schraudolph bit-trick on VectorE
       (fused scale+bias with f32->int16 output convert; the int16 bits
       ARE the bf16 probabilities) to balance the two engines. Band-edge
       triangles masked multiplicatively on VectorE (one strided op per
       block covering both 128x128 corners).
  PV (flipped): per 128-query block, lhsT = P^T slice (queries become
       output partitions) and rhs = [V64 | ones1], so each matmul has only
       N=65 output columns; the <=5 key-block contributions accumulate in
       one burst. Four query blocks pack into one [128, 4, 65] PSUM tile
       (a single bank), so PSUM holds 3 score slots + 2 PV tiles exactly.
  evac: one DVE copy per 4 query blocks (260 cols) PSUM -> SBUF outbuf;
       outputs ([q, V|denom] layout) ship eagerly per 8 query blocks.
       Softmax division happens on host.

All inputs bf16 (host-cast): Q^T/K^T packed on the same 64 partitions
(one tensor, DMA'd in quarters so the first blocks start early), V
interleaved [V|ones] per block on host. QK runs two blocks ahead of its
consumers to keep ACT/DVE fed; PE p-state warmup matmuls run during the
first input DMA. Non-final-head output DMAs issue from GpSimd (SWDGE)
so they cannot block input DMAs on the SP queue; the final head issues
per-tile from SP (idle at the tail, lower latency than SWDGE).
"""

import sys

sys.path.insert(0, "/opt/trn_rl_repo")

import numpy as np
import ml_dtypes

B, H, L, D = 2, 12, 4096, 64
W = 256            # one-sided window
NCORES = 8
HPC = (B * H) // NCORES   # heads per core = 3
BLK = 128                 # key block (partition dim)
NB = L // BLK             # 32 key blocks per head
SPAN = 2 * W + BLK        # 640 query columns per key block
CTXW = 512                # ctx psum tile width (1 bank)
NT = L // CTXW            # 8 ctx tiles per head

_CACHE = {}
REPEAT = 1           # duplicate compute body for wall-clock timing

# Key blocks whose exp runs as a one-op Schraudolph bit-trick on VectorE
# instead of ScalarE, rebalancing the two engines (ACT is the bottleneck):
# pt_bits_int16 = rint(score * 128*log2(e)/8 + 128*(127 - corr)), viewed
# as bf16. ~2-4% relative error on those probabilities; softmax-normalized
# output error stays ~1e-2 (tolerance 2e-2). Full-span middle blocks only.
DVE_EXP_BLOCKS = frozenset((3, 6, 10, 13, 17, 21, 24, 28))
SCHRAUDOLPH_C1 = 128.0 / np.log(2.0) / 8.0
SCHRAUDOLPH_C2 = 128.0 * (127.0 - 0.0579)

QK_AHEAD = 2         # blocks of QK lookahead (st pool depth - 1)


def _geometry():
    geo = []
    for kb in range(NB):
        K0 = BLK * kb
        qbase = K0 - W
        qlo = max(0, qbase)
        qhi = min(L, K0 + BLK + W)
        geo.append((K0, qbase, qlo, qhi))
    contrib = {t: [] for t in range(NT)}
    for kb, (K0, qbase, qlo, qhi) in enumerate(geo):
        for t in range(qlo // CTXW, (qhi - 1) // CTXW + 1):
            contrib[t].append(kb)
    last_kb = {t: kbs[-1] for t, kbs in contrib.items()}
    return geo, last_kb


def _build_program():
    import concourse.bacc as bacc
    import concourse.bass as bass
    import concourse.mybir as mybir
    import concourse.tile as tile

    f32 = mybir.dt.float32
    bf16 = mybir.dt.bfloat16

    nc = bacc.Bacc("TRN2", target_bir_lowering=False, debug=False)

    qk_d = nc.dram_tensor("qk", [HPC, D, 2 * L], bf16, kind="ExternalInput").ap()
    v_d = nc.dram_tensor("v", [HPC, BLK, NB * (D + 1)], bf16, kind="ExternalInput").ap()
    em_d = nc.dram_tensor("masks", [BLK, 2, BLK], bf16, kind="ExternalInput").ap()
    out_d = nc.dram_tensor("out", [HPC, BLK, NB * (D + 1)], f32, kind="ExternalOutput").ap()

    geo, last_kb = _geometry()

    with tile.TileContext(nc) as tc:
        with (
            tc.tile_pool(name="const", bufs=1) as constp,
            tc.tile_pool(name="qk", bufs=HPC + 1) as qkp,
            tc.tile_pool(name="v2", bufs=HPC + 1) as v2p,
            tc.tile_pool(name="outb", bufs=HPC) as outp,
            tc.tile_pool(name="pt", bufs=8) as ptp,
            tc.tile_pool(name="st", bufs=QK_AHEAD + 1, space="PSUM") as stp,
            tc.tile_pool(name="pv", bufs=2, space="PSUM") as pvp,
        ):
            em = constp.tile([BLK, 2, BLK], bf16)
            # zeroed scratch for PE p-state warmup matmuls
            ws = constp.tile([D, 512], bf16)
            nc.gpsimd.memset(ws, 0.0)

            first = True
            heads = [hh for _ in range(REPEAT) for hh in range(HPC)]
            for hi, h in enumerate(heads):
                last_head = hi == len(heads) - 1
                # packed [Q^T | K^T] on the same 64 partitions; DMA split in
                # quarters (interleaved with V halves) so the first blocks'
                # inputs land as early as possible
                qk = qkp.tile([D, 2, L], bf16, name="qk_t", tag="qk_t")
                qk_src = qk_d[h].rearrange("p (s l) -> p s l", s=2)
                # per-block [V64 | ones64] PV weights, interleaved on host
                v2 = v2p.tile([BLK, NB, D + 1], bf16, name="v2_t", tag="v2_t")
                v_src = v_d[h].rearrange("p (nb d) -> p nb d", nb=NB)

                Q = L // 4
                nc.sync.dma_start(out=qk[:, :, 0:Q], in_=qk_src[:, :, 0:Q])
                if first:
                    nc.sync.dma_start(out=em, in_=em_d)
                    first = False
                nc.sync.dma_start(out=v2[:, 0 : NB // 2], in_=v_src[:, 0 : NB // 2])
                nc.sync.dma_start(out=qk[:, :, Q : 2 * Q], in_=qk_src[:, :, Q : 2 * Q])
                nc.sync.dma_start(
                    out=qk[:, :, 2 * Q : 3 * Q], in_=qk_src[:, :, 2 * Q : 3 * Q]
                )
                nc.sync.dma_start(out=v2[:, NB // 2 :], in_=v_src[:, NB // 2 :])
                nc.sync.dma_start(out=qk[:, :, 3 * Q :], in_=qk_src[:, :, 3 * Q :])
                qt = qk[:, 0]
                kt = qk[:, 1]

                outbuf = outp.tile([BLK, NB * (D + 1)], f32, name="outb_t", tag="outb_t")

                pts = {}
                pending_evac = []

                def emit_evac(qg):
                    # one evac covers 4 packed query blocks (one PSUM bank)
                    pvt = pts.pop(("pv", qg))
                    c0 = 4 * qg * (D + 1)
                    nc.vector.tensor_copy(
                        outbuf[:, c0 : c0 + 4 * (D + 1)],
                        pvt.rearrange("p s d -> p (s d)"),
                    )
                    # ship finished output eagerly; GpSimd (SWDGE) for
                    # non-final heads so SP input DMAs are never blocked,
                    # SP per-chunk for the final head's exposed tail
                    step = 1 if last_head else 2
                    if (qg + 1) % step == 0:
                        d0 = 4 * (qg + 1 - step) * (D + 1)
                        d1 = 4 * (qg + 1) * (D + 1)
                        eng = nc.sync if last_head else nc.gpsimd
                        eng.dma_start(
                            out=out_d[h][:, d0:d1], in_=outbuf[:, d0:d1]
                        )

                sts = {}

                def emit_qk(kb):
                    st = stp.tile([BLK, SPAN], f32, name="st_t", tag="st_t")
                    if hi == 0 and kb == 0:
                        # ramp the PE p-state while the first input DMA is in
                        # flight; results land in a bank the real QK clears
                        # with start=True
                        for _ in range(5):
                            nc.tensor.matmul(
                                st[:, 0:512], ws[:, 0:BLK], ws, start=True, stop=True
                            )
                    K0, qbase, qlo, qhi = geo[kb]
                    a, b = qlo - qbase, qhi - qbase
                    lhsT = kt[:, K0 : K0 + BLK]
                    if a < 512:
                        m = min(512, b)
                        nc.tensor.matmul(
                            st[:, a:m],
                            lhsT,
                            qt[:, qbase + a : qbase + m],
                            start=True,
                            stop=True,
                        )
                    if b > 512:
                        nc.tensor.matmul(
                            st[:, 512:b],
                            lhsT,
                            qt[:, qbase + 512 : qbase + b],
                            start=True,
                            stop=True,
                        )
                    sts[kb] = (st, a, b)

                for kb in range(QK_AHEAD):
                    emit_qk(kb)
                scale = float(1.0 / np.sqrt(D))
                Exp = mybir.ActivationFunctionType.Exp
                for kb in range(NB + 2):
                    if kb < NB:
                        if kb + QK_AHEAD < NB:
                            emit_qk(kb + QK_AHEAD)  # QK ahead: PE priority bias
                        st, a, b = sts.pop(kb)
                        pt = ptp.tile([BLK, SPAN], bf16, name="pt_t", tag="pt_t")
                        pts[kb] = pt

                        # exp(S/8)
                        if (kb - hi) % 32 in DVE_EXP_BLOCKS and 2 <= kb <= 29:
                            # Schraudolph exp on DVE: fused scale+bias with
                            # f32->int16 output convert; int16 bits ARE the
                            # bf16 probabilities
                            nc.vector.tensor_scalar(
                                pt.bitcast(mybir.dt.int16),
                                st,
                                float(SCHRAUDOLPH_C1),
                                float(SCHRAUDOLPH_C2),
                                mybir.AluOpType.mult,
                                mybir.AluOpType.add,
                            )
                        else:
                            nc.scalar.activation(
                                pt[:, a:b], st[:, a:b], Exp, scale=scale
                            )

                        # triangle masks on the band edges (multiplicative);
                        # middle blocks hit both corners in one strided op
                        if a == 0 and b == SPAN:
                            pte = bass.AP(
                                tensor=pt.tensor,
                                offset=pt.offset,
                                ap=[pt.ap[0], [4 * BLK, 2], [1, BLK]],
                            )
                            nc.vector.tensor_mul(pte, pte, em)
                        elif a == 0:
                            nc.vector.tensor_mul(
                                pt[:, 0:BLK], pt[:, 0:BLK], em[:, 0, :]
                            )
                        elif b == SPAN:
                            nc.vector.tensor_mul(
                                pt[:, 512:SPAN], pt[:, 512:SPAN], em[:, 1, :]
                            )

                    # evacs deferred one block: they wait on PV, and emitting
                    # them here keeps them from head-of-line-blocking this
                    # block's mask in DVE's in-order queue
                    for t in pending_evac:
                        emit_evac(t)
                    pending_evac = []

                    # flipped PV: one [128 queries, V|1] tile per query block,
                    # accumulated over its <=5 key-block contributors in one
                    # burst (lhsT = P^T slice, N = 65 output columns)
                    qb = kb - 2
                    if qb >= 0:
                        qg = qb // 4
                        if qb % 4 == 0:
                            pts[("pv", qg)] = pvp.tile(
                                [BLK, 4, D + 1], f32, name="pv_t", tag="pv_t"
                            )
                        pvt = pts[("pv", qg)][:, qb % 4, :]
                        kls = [
                            k for k in range(max(0, qb - 2), min(NB - 1, qb + 2) + 1)
                        ]
                        for j, kp in enumerate(kls):
                            i = qb - kp + 2
                            nc.tensor.matmul(
                                pvt,
                                pts[kp][:, i * BLK : (i + 1) * BLK],
                                v2[:, kp, :],
                                start=(j == 0),
                                stop=(j == len(kls) - 1),
                            )
                        if qb % 4 == 3:
                            pending_evac.append(qg)
                        # pt(qb-2) has no further readers; drop our handle
                        if qb - 2 >= 0:
                            pts.pop(qb - 2, None)

                for t in pending_evac:
                    emit_evac(t)
                pending_evac = []

    nc.compile()
    return nc


def _get_nc():
    if "nc" not in _CACHE:
        _CACHE["nc"] = _build_program()
    return _CACHE["nc"]


def _host_prep(q, k, v):
    bf = ml_dtypes.bfloat16
    qf = np.asarray(q, dtype=np.float32).reshape(B * H, L, D)
    kf = np.asarray(k, dtype=np.float32).reshape(B * H, L, D)
    # packed [Q^T | K^T] on the same 64 partitions: (BH, 64, 2*L)
    qk = np.empty((B * H, D, 2 * L), dtype=bf)
    qk[:, :, 0:L] = qf.transpose(0, 2, 1).astype(bf)
    qk[:, :, L : 2 * L] = kf.transpose(0, 2, 1).astype(bf)
    # PV weights per key block: [V64 | ones1], (BH, BLK, NB*(D+1)) with
    # [r, kb, d] = v[kb*BLK + r, d] and [r, kb, D] = 1
    vf = np.ones((B * H, BLK, NB, D + 1), dtype=bf)
    vf[:, :, :, 0:D] = (
        np.asarray(v, dtype=np.float32)
        .reshape(B * H, NB, BLK, D)
        .transpose(0, 2, 1, 3)
        .astype(bf)
    )
    vf = vf.reshape(B * H, BLK, NB * (D + 1))

    i = np.arange(BLK)
    em = np.zeros((BLK, 2, BLK), dtype=bf)
    em[:, 0, :] = (i[None, :] >= i[:, None]).astype(bf)  # left: col>=row
    em[:, 1, :] = (i[None, :] <= i[:, None]).astype(bf)  # right: col<=row

    in_maps = []
    for c in range(NCORES):
        sl = slice(c * HPC, (c + 1) * HPC)
        in_maps.append(
            {
                "qk": np.ascontiguousarray(qk[sl]),
                "v": np.ascontiguousarray(vf[sl]),
                "masks": em,
            }
        )
    return in_maps


def kernel(q, k, v, padding_mask):
    from concourse.bass_utils import run_bass_kernel_spmd

    pm = np.asarray(padding_mask)
    assert pm.all(), "kernel specialized for all-ones padding mask"

    nc = _get_nc()
    in_maps = _host_prep(q, k, v)
    try:
        res = run_bass_kernel_spmd(nc, in_maps, core_ids=list(range(NCORES)))
    except Exception:
        # transient NRT_EXEC_UNIT_UNRECOVERABLE has been observed once on
        # this axon setup; the identical program+inputs passed on retry
        res = run_bass_kernel_spmd(nc, in_maps, core_ids=list(range(NCORES)))
    arr = np.concatenate(
        [res.results[c]["out"] for c in range(NCORES)], axis=0
    ).reshape(B * H, BLK, NB, D + 1)  # [bh, q%128, qb, V|den]
    full = arr[:, :, :, 0:D] / arr[:, :, :, D : D + 1]  # softmax norm on host
    out = full.transpose(0, 2, 1, 3).reshape(B, H, L, D)
    return np.ascontiguousarray(out.astype(np.float32))


# revision 7
# speedup vs baseline: 1.1279x; 1.0098x over previous
"""Longformer sliding-window attention on 8 trn2 NeuronCores.

B=2, H=12, L=4096, D=64, one-sided window w=256 (full window 513).
Shard: 24 (b,h) pairs -> 3 heads per core.

Per-core algorithm (per head, 32 key-blocks of 128 keys):
  S^T: per key block, scores (128 keys x <=640 queries) via bf16 matmuls
       (lhsT = K^T block, rhs = Q^T span) into a [128, 640] PSUM tile
       (2 banks, triple-buffered), split at the PSUM bank boundary.
  P^T = exp(S/8) -> SBUF bf16. Most blocks run on ScalarE (activation);
       a spread subset runs as a one-op Schraudolph bit-trick on VectorE
       (fused scale+bias with f32->int16 output convert; the int16 bits
       ARE the bf16 probabilities) to balance the two engines. Band-edge
       triangles masked multiplicatively on VectorE (one strided op per
       block covering both 128x128 corners).
  PV (flipped): per 128-query block, lhsT = P^T slice (queries become
       output partitions) and rhs = [V64 | ones1], so each matmul has only
       N=65 output columns; the <=5 key-block contributions accumulate in
       one burst. Four query blocks pack into one [128, 4, 65] PSUM tile
       (a single bank), so PSUM holds 3 score slots + 2 PV tiles exactly.
  evac: one DVE copy per 4 query blocks (260 cols) PSUM -> SBUF outbuf;
       outputs ([q, V|denom] layout) ship eagerly per 8 query blocks.
       Softmax division happens on host.

All inputs bf16 (host-cast): Q^T/K^T packed on the same 64 partitions
(one tensor, DMA'd in quarters so the first blocks start early), V
interleaved [V|ones] per block on host. QK runs two blocks ahead of its
consumers to keep ACT/DVE fed; PE p-state warmup matmuls run during the
first input DMA. Non-final-head output DMAs issue from GpSimd (SWDGE)
so they cannot block input DMAs on the SP queue; the final head issues
per-tile from SP (idle at the tail, lower latency than SWDGE).
"""

import sys

sys.path.insert(0, "/opt/trn_rl_repo")

import numpy as np
import ml_dtypes

B, H, L, D = 2, 12, 4096, 64
W = 256            # one-sided window
NCORES = 8
HPC = (B * H) // NCORES   # heads per core = 3
BLK = 128                 # key block (partition dim)
NB = L // BLK             # 32 key blocks per head
SPAN = 2 * W + BLK        # 640 query columns per key block
CTXW = 512                # ctx psum tile width (1 bank)
NT = L // CTXW            # 8 ctx tiles per head

_CACHE = {}
REPEAT = 1           # duplicate compute body for wall-clock timing

# Key blocks whose exp runs as a one-op Schraudolph bit-trick on VectorE
# instead of ScalarE, rebalancing the two engines (ACT is the bottleneck):
# pt_bits_int16 = rint(score * 128*log2(e)/8 + 128*(127 - corr)), viewed
# as bf16. ~2-4% relative error on those probabilities; softmax-normalized
# output error stays ~1e-2 (tolerance 2e-2). Full-span middle blocks only.
DVE_EXP_BLOCKS = frozenset((2, 5, 9, 13, 16, 20, 24, 28))
SCHRAUDOLPH_C1 = 128.0 / np.log(2.0) / 8.0
SCHRAUDOLPH_C2 = 128.0 * (127.0 - 0.0579)

QK_AHEAD = 2         # blocks of QK lookahead (st pool depth - 1)


def _geometry():
    geo = []
    for kb in range(NB):
        K0 = BLK * kb
        qbase = K0 - W
        qlo = max(0, qbase)
        qhi = min(L, K0 + BLK + W)
        geo.append((K0, qbase, qlo, qhi))
    contrib = {t: [] for t in range(NT)}
    for kb, (K0, qbase, qlo, qhi) in enumerate(geo):
        for t in range(qlo // CTXW, (qhi - 1) // CTXW + 1):
            contrib[t].append(kb)
    last_kb = {t: kbs[-1] for t, kbs in contrib.items()}
    return geo, last_kb


def _build_program():
    import concourse.bacc as bacc
    import concourse.bass as bass
    import concourse.mybir as mybir
    import concourse.tile as tile

    f32 = mybir.dt.float32
    bf16 = mybir.dt.bfloat16

    nc = bacc.Bacc("TRN2", target_bir_lowering=False, debug=False)

    qk_d = nc.dram_tensor("qk", [HPC, D, 2 * L], bf16, kind="ExternalInput").ap()
    v_d = nc.dram_tensor("v", [HPC, BLK, NB * (D + 1)], bf16, kind="ExternalInput").ap()
    em_d = nc.dram_tensor("masks", [BLK, 2, BLK], bf16, kind="ExternalInput").ap()
    out_d = nc.dram_tensor("out", [HPC, BLK, NB * (D + 1)], f32, kind="ExternalOutput").ap()

    geo, last_kb = _geometry()

    with tile.TileContext(nc) as tc:
        with (
            tc.tile_pool(name="const", bufs=1) as constp,
            tc.tile_pool(name="qk", bufs=HPC + 1) as qkp,
            tc.tile_pool(name="v2", bufs=HPC + 1) as v2p,
            tc.tile_pool(name="outb", bufs=HPC) as outp,
            tc.tile_pool(name="pt", bufs=8) as ptp,
            tc.tile_pool(name="st", bufs=QK_AHEAD + 1, space="PSUM") as stp,
            tc.tile_pool(name="pv", bufs=2, space="PSUM") as pvp,
        ):
            em = constp.tile([BLK, 2, BLK], bf16)
            # zeroed scratch for PE p-state warmup matmuls
            ws = constp.tile([D, 512], bf16)
            nc.gpsimd.memset(ws, 0.0)

            first = True
            heads = [hh for _ in range(REPEAT) for hh in range(HPC)]
            for hi, h in enumerate(heads):
                last_head = hi == len(heads) - 1
                # packed [Q^T | K^T] on the same 64 partitions; DMA split in
                # quarters (interleaved with V halves) so the first blocks'
                # inputs land as early as possible
                qk = qkp.tile([D, 2, L], bf16, name="qk_t", tag="qk_t")
                qk_src = qk_d[h].rearrange("p (s l) -> p s l", s=2)
                # per-block [V64 | ones64] PV weights, interleaved on host
                v2 = v2p.tile([BLK, NB, D + 1], bf16, name="v2_t", tag="v2_t")
                v_src = v_d[h].rearrange("p (nb d) -> p nb d", nb=NB)

                Q = L // 4
                nc.sync.dma_start(out=qk[:, :, 0:Q], in_=qk_src[:, :, 0:Q])
                if first:
                    nc.sync.dma_start(out=em, in_=em_d)
                    first = False
                nc.sync.dma_start(out=v2[:, 0 : NB // 2], in_=v_src[:, 0 : NB // 2])
                nc.sync.dma_start(out=qk[:, :, Q : 2 * Q], in_=qk_src[:, :, Q : 2 * Q])
                nc.sync.dma_start(
                    out=qk[:, :, 2 * Q : 3 * Q], in_=qk_src[:, :, 2 * Q : 3 * Q]
                )
                nc.sync.dma_start(out=v2[:, NB // 2 :], in_=v_src[:, NB // 2 :])
                nc.sync.dma_start(out=qk[:, :, 3 * Q :], in_=qk_src[:, :, 3 * Q :])
                qt = qk[:, 0]
                kt = qk[:, 1]

                outbuf = outp.tile([BLK, NB * (D + 1)], f32, name="outb_t", tag="outb_t")

                pts = {}
                pending_evac = []

                def emit_evac(qg):
                    # one evac covers 4 packed query blocks (one PSUM bank)
                    pvt = pts.pop(("pv", qg))
                    c0 = 4 * qg * (D + 1)
                    nc.vector.tensor_copy(
                        outbuf[:, c0 : c0 + 4 * (D + 1)],
                        pvt.rearrange("p s d -> p (s d)"),
                    )
                    # ship finished output eagerly; GpSimd (SWDGE) for
                    # non-final heads so SP input DMAs are never blocked,
                    # SP per-chunk for the final head's exposed tail
                    step = 1 if last_head else 2
                    if (qg + 1) % step == 0:
                        d0 = 4 * (qg + 1 - step) * (D + 1)
                        d1 = 4 * (qg + 1) * (D + 1)
                        eng = nc.sync if last_head else nc.gpsimd
                        eng.dma_start(
                            out=out_d[h][:, d0:d1], in_=outbuf[:, d0:d1]
                        )

                sts = {}

                def emit_qk(kb):
                    st = stp.tile([BLK, SPAN], f32, name="st_t", tag="st_t")
                    if hi == 0 and kb == 0:
                        # ramp the PE p-state while the first input DMA is in
                        # flight; results land in a bank the real QK clears
                        # with start=True
                        for _ in range(5):
                            nc.tensor.matmul(
                                st[:, 0:512], ws[:, 0:BLK], ws, start=True, stop=True
                            )
                    K0, qbase, qlo, qhi = geo[kb]
                    a, b = qlo - qbase, qhi - qbase
                    lhsT = kt[:, K0 : K0 + BLK]
                    if a < 512:
                        m = min(512, b)
                        nc.tensor.matmul(
                            st[:, a:m],
                            lhsT,
                            qt[:, qbase + a : qbase + m],
                            start=True,
                            stop=True,
                        )
                    if b > 512:
                        nc.tensor.matmul(
                            st[:, 512:b],
                            lhsT,
                            qt[:, qbase + 512 : qbase + b],
                            start=True,
                            stop=True,
                        )
                    sts[kb] = (st, a, b)

                for kb in range(QK_AHEAD):
                    emit_qk(kb)
                scale = float(1.0 / np.sqrt(D))
                Exp = mybir.ActivationFunctionType.Exp
                for kb in range(NB + 2):
                    if kb < NB:
                        if kb + QK_AHEAD < NB:
                            emit_qk(kb + QK_AHEAD)  # QK ahead: PE priority bias
                        st, a, b = sts.pop(kb)
                        pt = ptp.tile([BLK, SPAN], bf16, name="pt_t", tag="pt_t")
                        pts[kb] = pt

                        # exp(S/8)
                        if (kb - hi) % 32 in DVE_EXP_BLOCKS and 2 <= kb <= 29:
                            # Schraudolph exp on DVE: fused scale+bias with
                            # f32->int16 output convert; int16 bits ARE the
                            # bf16 probabilities
                            nc.vector.tensor_scalar(
                                pt.bitcast(mybir.dt.int16),
                                st,
                                float(SCHRAUDOLPH_C1),
                                float(SCHRAUDOLPH_C2),
                                mybir.AluOpType.mult,
                                mybir.AluOpType.add,
                            )
                        else:
                            nc.scalar.activation(
                                pt[:, a:b], st[:, a:b], Exp, scale=scale
                            )

                        # triangle masks on the band edges (multiplicative);
                        # middle blocks hit both corners in one strided op
                        if a == 0 and b == SPAN:
                            pte = bass.AP(
                                tensor=pt.tensor,
                                offset=pt.offset,
                                ap=[pt.ap[0], [4 * BLK, 2], [1, BLK]],
                            )
                            nc.vector.tensor_mul(pte, pte, em)
                        elif a == 0:
                            nc.vector.tensor_mul(
                                pt[:, 0:BLK], pt[:, 0:BLK], em[:, 0, :]
                            )
                        elif b == SPAN:
                            nc.vector.tensor_mul(
                                pt[:, 512:SPAN], pt[:, 512:SPAN], em[:, 1, :]
                            )

                    # evacs deferred one block: they wait on PV, and emitting
                    # them here keeps them from head-of-line-blocking this
                    # block's mask in DVE's in-order queue
                    for t in pending_evac:
                        emit_evac(t)
                    pending_evac = []

                    # flipped PV: one [128 queries, V|1] tile per query block,
                    # accumulated over its <=5 key-block contributors in one
                    # burst (lhsT = P^T slice, N = 65 output columns)
                    qb = kb - 2
                    if qb >= 0:
                        qg = qb // 4
                        if qb % 4 == 0:
                            pts[("pv", qg)] = pvp.tile(
                                [BLK, 4, D + 1], f32, name="pv_t", tag="pv_t"
                            )
                        pvt = pts[("pv", qg)][:, qb % 4, :]
                        kls = [
                            k for k in range(max(0, qb - 2), min(NB - 1, qb + 2) + 1)
                        ]
                        for j, kp in enumerate(kls):
                            i = qb - kp + 2
                            nc.tensor.matmul(
                                pvt,
                                pts[kp][:, i * BLK : (i + 1) * BLK],
                                v2[:, kp, :],
                                start=(j == 0),
                                stop=(j == len(kls) - 1),
                            )
                        if qb % 4 == 3:
                            pending_evac.append(qg)
                        # pt(qb-2) has no further readers; drop our handle
                        if qb - 2 >= 0:
                            pts.pop(qb - 2, None)

                for t in pending_evac:
                    emit_evac(t)
                pending_evac = []

    nc.compile()
    return nc


def _get_nc():
    if "nc" not in _CACHE:
        _CACHE["nc"] = _build_program()
    return _CACHE["nc"]


def _host_prep(q, k, v):
    bf = ml_dtypes.bfloat16
    qf = np.asarray(q, dtype=np.float32).reshape(B * H, L, D)
    kf = np.asarray(k, dtype=np.float32).reshape(B * H, L, D)
    # packed [Q^T | K^T] on the same 64 partitions: (BH, 64, 2*L)
    qk = np.empty((B * H, D, 2 * L), dtype=bf)
    qk[:, :, 0:L] = qf.transpose(0, 2, 1).astype(bf)
    qk[:, :, L : 2 * L] = kf.transpose(0, 2, 1).astype(bf)
    # PV weights per key block: [V64 | ones1], (BH, BLK, NB*(D+1)) with
    # [r, kb, d] = v[kb*BLK + r, d] and [r, kb, D] = 1
    vf = np.ones((B * H, BLK, NB, D + 1), dtype=bf)
    vf[:, :, :, 0:D] = (
        np.asarray(v, dtype=np.float32)
        .reshape(B * H, NB, BLK, D)
        .transpose(0, 2, 1, 3)
        .astype(bf)
    )
    vf = vf.reshape(B * H, BLK, NB * (D + 1))

    i = np.arange(BLK)
    em = np.zeros((BLK, 2, BLK), dtype=bf)
    em[:, 0, :] = (i[None, :] >= i[:, None]).astype(bf)  # left: col>=row
    em[:, 1, :] = (i[None, :] <= i[:, None]).astype(bf)  # right: col<=row

    in_maps = []
    for c in range(NCORES):
        sl = slice(c * HPC, (c + 1) * HPC)
        in_maps.append(
            {
                "qk": np.ascontiguousarray(qk[sl]),
                "v": np.ascontiguousarray(vf[sl]),
                "masks": em,
            }
        )
    return in_maps


def kernel(q, k, v, padding_mask):
    from concourse.bass_utils import run_bass_kernel_spmd

    pm = np.asarray(padding_mask)
    assert pm.all(), "kernel specialized for all-ones padding mask"

    nc = _get_nc()
    in_maps = _host_prep(q, k, v)
    try:
        res = run_bass_kernel_spmd(nc, in_maps, core_ids=list(range(NCORES)))
    except Exception:
        # transient NRT_EXEC_UNIT_UNRECOVERABLE has been observed once on
        # this axon setup; the identical program+inputs passed on retry
        res = run_bass_kernel_spmd(nc, in_maps, core_ids=list(range(NCORES)))
    arr = np.concatenate(
        [res.results[c]["out"] for c in range(NCORES)], axis=0
    ).reshape(B * H, BLK, NB, D + 1)  # [bh, q%128, qb, V|den]
    full = arr[:, :, :, 0:D] / arr[:, :, :, D : D + 1]  # softmax norm on host
    out = full.transpose(0, 2, 1, 3).reshape(B, H, L, D)
    return np.ascontiguousarray(out.astype(np.float32))
